# revision 31
# baseline (speedup 1.0000x reference)
"""Trainium2 Bass kernel for nn_Attn: per-sample neighbor attention softmax.

Math: reference computes
    temp[b]   = encoder_outputs[b, current_index]              # [64]
    energy    = enc_nb @ W.T + bias                            # [B, N, 64]
    logits    = einsum('bnd,bd->bn', energy, temp)             # [B, N]
    out       = softmax(logits, axis=1)

Algebraic rewrite:
    logits[b, n] = enc_nb[b, n] . v[b],   v[b] = W.T @ temp[b]
(The bias term is constant over n per sample, so it cancels in softmax.)

This version runs the matvec on the PE (tensor engine). The host ships
enc_nb TRANSPOSED per sample (s-major, [16, 512, 2048] fp16 per core),
so the contraction dim s lands on SBUF partitions and the PE can
contract it natively — no transposes on device, and the streaming DMA
stays fully contiguous (4 KB per partition per sample). fp16 halves HBM
traffic vs fp32 (measured end-to-end rel err ~1.8e-3 vs the 2e-2
budget; products accumulate in fp32 PSUM).

Why not the DVE: a fused multiply+reduce (scalar_tensor_tensor /
tensor_tensor_reduce) only has a 1x micro-op program (~600 ns per
[128, 512] column -> 154 us/core), and ACT-side reductions measure
~1 us/op. The PE does the same contraction in 16 matmuls per sample
(~4 us/core-sample) leaving every other engine far below the ~90 us
DMA roofline for the 32 MB/core stream.

Per-core structure (16 samples):
  VT build: 4 fp32 matmuls W_chunk.T @ tempT -> psum [128, 16], cast
      to fp16 VT_q tiles (v transposed, s-chunk on partitions).
  Per sample b:
      two 1 MB half-loads -> [128, 2, 2048] fp16 each (partition p
          holds rows {q*128+p} of enc_t[b]; halving lets the PE start
          after half the sample has landed)
      q-major accumulating matmuls into 4 open psum groups:
          psum_nb[16, 512] += VT_q.T @ half[:, q, nb]  (row b is sample
          b's scores; other 15 rows are cross-sample garbage, computed
          for free since PE time is column-count-bound)
      ACT: exp (no max subtraction: |logit| < 50 << 88, fp32 exp is
          safe) + free-dim accumulate -> partials[16, nb]
      DVE: reduce partials -> 1/sumexp; tensor_scalar scales the whole
          [16, 2048] block (free-dim-bound, so the 15 garbage rows cost
          nothing)
      one 8 KB store of row b (DMA extracts the row; engine APs can
          only start at partitions 0/32/64/96)
"""

from contextlib import ExitStack

import numpy as np

import concourse.bacc as bacc
import concourse.bass as bass
import concourse.mybir as mybir
import concourse.tile as tile
from concourse.bass_utils import run_bass_kernel_spmd

N_CORES = 8
B = 128          # batch
N = 2048         # neighbors per sample
S0 = 512         # neighbor feature dim
D = 64           # query feature dim
BC = B // N_CORES  # samples per core = 16
Q = S0 // 128    # s-chunks (contraction tiles) = 4
NB = N // 512    # n-blocks (psum column tiles) = 4
LOAD_BUFS = 8    # half-sample load tiles in flight (1 MB each)
FP32 = mybir.dt.float32
FP16 = mybir.dt.float16


def _emit(ctx: ExitStack, tc: "tile.TileContext", enc, wt, out):
    nc = tc.nc
    const_pool = ctx.enter_context(tc.tile_pool(name="const", bufs=1))
    load_pool = ctx.enter_context(tc.tile_pool(name="load", bufs=LOAD_BUFS))
    vt_psum = ctx.enter_context(tc.tile_pool(name="vtps", bufs=2, space="PSUM"))
    sc_psum = ctx.enter_context(tc.tile_pool(name="scps", bufs=6, space="PSUM"))
    es_pool = ctx.enter_context(tc.tile_pool(name="es", bufs=3))
    st_pool = ctx.enter_context(tc.tile_pool(name="st", bufs=4))

    # VT_q[p, b] = sum_d W[d, q*128+p] * temp[b, d]  == v[b] transposed,
    # s-chunk q on partitions. W and tempT arrive packed in one [64, 528]
    # input so each matmul's weight load depends on exactly one DMA.
    # scalar ring: lets the first enc load start immediately on SP
    wt_sb = const_pool.tile([D, S0 + BC], FP32)
    nc.scalar.dma_start(wt_sb[:], wt[:])
    vt16 = []
    for q in range(Q):
        vt_ps = vt_psum.tile([128, BC], FP32, tag="vtps")
        nc.tensor.matmul(
            vt_ps[:], wt_sb[:, q * 128 : (q + 1) * 128], wt_sb[:, S0:]
        )
        v = const_pool.tile([128, BC], FP16, tag=f"vt{q}")
        nc.vector.tensor_copy(v[:], vt_ps[:])
        vt16.append(v)

    pr_pool = ctx.enter_context(tc.tile_pool(name="pr", bufs=3))

    # [16, 512, 2048] -> per sample [p=128, q=4, n=2048], loaded as two
    # 1 MB halves (q01, q23) so the PE can start on a sample after half
    # its data has landed (shorter fill and drain of the pipeline).
    enc_r = enc.rearrange("b (q p) n -> b p q n", q=Q)

    # Engine APs may only start at partition 0/32/64/96, so row b of the
    # scaled [16, N] block is extracted by the store DMA (DMA descriptors
    # address any partition). Stores are emitted with a 2-sample lag so
    # the SP sequencer never blocks a load doorbell on an unfinished
    # DVE scale (HWDGE rings are FIFO per issuing engine).
    pending_stores = []

    def flush_store():
        # scalar-ring HWDGE: keeps the 16 stores off the SP ring, which
        # carries the thirty-two 1 MB loads (the critical DMA track).
        b0, prb = pending_stores.pop(0)
        nc.scalar.dma_start(out[b0 : b0 + 1, :], prb[b0 : b0 + 1, :])

    for b in range(BC):
        halves = []
        for h in range(2):
            eth = load_pool.tile([128, 2, N], FP16, tag=f"load{h}")
            nc.sync.dma_start(eth[:], enc_r[b, :, 2 * h : 2 * h + 2, :])
            halves.append(eth)
        partials = st_pool.tile([BC, NB], FP32, tag="partials")
        # q-major matmul order: all four psum accumulation groups stay
        # open across the two half-loads (interleaved groups are fine on
        # HW: accumulate is per-address; the bass group check is skipped)
        pss = [
            sc_psum.tile([BC, 512], FP32, tag="scores", name=f"ps_{b}_{i}")
            for i in range(NB)
        ]
        for q in range(Q):
            for nb in range(NB):
                nc.tensor.matmul(
                    pss[nb][:],
                    vt16[q][:],
                    halves[q // 2][:, q % 2, nb * 512 : (nb + 1) * 512],
                    start=(q == 0),
                    stop=(q == Q - 1),
                    skip_group_check=True,
                )
        eS = es_pool.tile([BC, N], FP32, tag="exp")
        for nb in range(NB):
            nc.scalar.activation(
                out=eS[:, nb * 512 : (nb + 1) * 512],
                in_=pss[nb][:],
                func=mybir.ActivationFunctionType.Exp,
                scale=1.0,
                accum_out=partials[:, nb : nb + 1],
            )
        sums = st_pool.tile([BC, 1], FP32, tag="sums")
        nc.vector.tensor_reduce(
            out=sums[:], in_=partials[:], axis=mybir.AxisListType.X,
            op=mybir.AluOpType.add,
        )
        recip = st_pool.tile([BC, 1], FP32, tag="recip")
        nc.vector.reciprocal(recip[:], sums[:])
        # one scale over the whole sample (cost is free-dim-bound; only
        # row b is real)
        prb = pr_pool.tile([BC, N], FP32)
        nc.vector.tensor_scalar(
            out=prb[:],
            in0=eS[:],
            scalar1=recip[:],
            scalar2=None,
            op0=mybir.AluOpType.mult,
        )
        pending_stores.append((b, prb))
        if len(pending_stores) > 2:
            flush_store()

    while pending_stores:
        flush_store()


_NC_CACHE = {}


def build_bass(reps: int = 1) -> bass.Bass:
    """reps>1 emits the body that many times in one NEFF (used by the
    timing harness to cancel per-dispatch overhead)."""
    if reps in _NC_CACHE:
        return _NC_CACHE[reps]
    nc = bacc.Bacc(trn_type="TRN2", target_bir_lowering=False, debug=False)
    enc = nc.dram_tensor("enc", [BC, S0, N], FP16, kind="ExternalInput").ap()
    wt = nc.dram_tensor("wt", [D, S0 + BC], FP32, kind="ExternalInput").ap()
    out = nc.dram_tensor("out", [BC, N], FP32, kind="ExternalOutput").ap()
    with tile.TileContext(nc) as tc:
        for _ in range(reps):
            with ExitStack() as ctx:
                _emit(ctx, tc, enc, wt, out)
    nc.compile()
    _NC_CACHE[reps] = nc
    return nc


def make_in_maps(inputs: dict) -> list[dict]:
    enc_out = np.ascontiguousarray(np.asarray(inputs["encoder_outputs"], dtype=np.float32))
    enc_nb = np.asarray(inputs["encoder_outputs_neighbor"], dtype=np.float32)
    w = np.ascontiguousarray(np.asarray(inputs["W"], dtype=np.float32))
    idx = int(np.asarray(inputs["current_index"]))
    temp = enc_out[:, idx, :]  # [B, D]

    in_maps = []
    for c in range(N_CORES):
        tb = temp[c * BC : (c + 1) * BC]  # [16, 64]
        wt = np.ascontiguousarray(
            np.concatenate([w, tb.T], axis=1), dtype=np.float32
        )
        enc_t = np.ascontiguousarray(
            enc_nb[c * BC : (c + 1) * BC].astype(np.float16).transpose(0, 2, 1)
        )
        in_maps.append({"enc": enc_t, "wt": wt})
    return in_maps


def kernel(**inputs) -> np.ndarray:
    nc = build_bass()
    in_maps = make_in_maps(inputs)
    res = run_bass_kernel_spmd(nc, in_maps, core_ids=list(range(N_CORES)))
    return np.concatenate([res.results[c]["out"] for c in range(N_CORES)], axis=0)
